# revision 25
# baseline (speedup 1.0000x reference)
"""Multi-head attention kernel for 8 Trainium2 NeuronCores.

Problem: B=2, S=2048, H=8, DK=DV=64, D=512 (nn_MultiHeadAttention).

Sharding: core c owns batch b=c//4 and query rows [512*r, 512*r+512) with
r = c%4. No collectives: each core recomputes the full K/V projections for
its batch locally (the ~25us of redundant PE work is far cheaper than the
barrier + AllGather latency it replaces).

Per-core device kernel, software-pipelined so the PE FIFO never waits on
the exp: round k issues scores(k) then the o-matmuls of round k-1, so each
o only executes after its attention tile finished exp ~1 round earlier.
  QT[p]   = wq2[p].T @ qT + bq              [128, 512]
  KT[p]   = wk2[p].T @ kT + bk              [128, 2048]
  V'[t]   = vT(t).T @ wv | ones col         [128, 8, 65]  (65th column of
            ones makes the o-matmul emit the softmax denominator in row 64)
  scoresT = KT[p] halves @ QT[p]            [128, 2, 512] per (pair, t),
                                            2 concurrent row-group matmuls
  attnT   = exp(scoresT / 8)                ScalarE, f16, no max-subtract
  o65[h] += V'[t,h] @ attnT[:, i]           accumulated over t; row 64 =
                                            softmax denominator
  bc      = ones(1x64).T @ denom row        K=1 matmul partition-broadcast
  rbc     = reciprocal_approx_fast(bc)      one DVE op per pair
  o2T[h]  = o65[h][0:64] * rbc              DVE, f16
  out     = sum_h o2T[h].T-slices @ wo[h] + bo'
bv is folded into the output bias on the host (bo' = bo + concat(bv) @ wo),
so the V projection needs no bias add on device.
"""

import numpy as np

B, S, H, DK, DV = 2, 2048, 8, 64, 64
D = H * DV  # 512
NCORES = 8
GROUP = 4  # cores per batch
ROWS = (B * S) // NCORES  # 512 query rows per core
NPAIR = H // 2  # 4 head pairs
NTT = S // 128  # 16 key/value tiles
NDC = D // 128  # 4 contraction chunks
P = 128
VW = DV + 1  # 65: V columns per head incl. the ones column

_prog = {}


def _build_program(repeats=1, hw_loop=0):
    from contextlib import ExitStack
    import contextlib

    import concourse.mybir as mybir
    import concourse.tile as tile
    from concourse import bacc

    f32 = mybir.dt.float32
    f16 = mybir.dt.float16  # fp16 PE datapath: separate+fast weight loads
    Exp = mybir.ActivationFunctionType.Exp

    nc = bacc.Bacc("TRN2", target_bir_lowering=False, debug=False, num_devices=NCORES)

    # DRAM I/O (per-core data; same program on all 8 cores)
    qt_d = nc.dram_tensor("qt", [P, NDC, ROWS], f16, kind="ExternalInput").ap()
    kt_d = nc.dram_tensor("kt", [S // 512, P, NDC, 512], f16, kind="ExternalInput").ap()
    vt_d = nc.dram_tensor("vt", [NTT // 4, P, 4, NDC, 128], f16, kind="ExternalInput").ap()
    wq_d = nc.dram_tensor("wq", [P, NDC, D], f16, kind="ExternalInput").ap()
    wk_d = nc.dram_tensor("wk", [P, NDC, D], f16, kind="ExternalInput").ap()
    wv_d = nc.dram_tensor("wv", [P, NDC, D], f16, kind="ExternalInput").ap()
    wo_d = nc.dram_tensor("wo", [DV, H, D], f16, kind="ExternalInput").ap()
    bqk_d = nc.dram_tensor("bqk", [P, 2 * NPAIR], f32, kind="ExternalInput").ap()
    bob_d = nc.dram_tensor("bob", [P, D], f32, kind="ExternalInput").ap()
    out_d = nc.dram_tensor("out", [ROWS // P, P, D], f32, kind="ExternalOutput").ap()

    with tile.TileContext(nc) as tc, ExitStack() as ctx:
        weights = ctx.enter_context(tc.tile_pool(name="weights", bufs=1))
        raw = ctx.enter_context(tc.tile_pool(name="raw", bufs=1))
        acts = ctx.enter_context(tc.tile_pool(name="acts", bufs=1))
        attn_pool = ctx.enter_context(tc.tile_pool(name="attn", bufs=4))
        small = ctx.enter_context(tc.tile_pool(name="small", bufs=2))
        # PSUM: tag "sc" 2 bufs x [128,2,512]f32 (2 banks each) dedicated to
        # scores (strict double-buffer against the exp), tag "pp" 2 bufs x
        # 1 bank for projection groups + denom broadcasts (double-buffered so
        # proj group k+1 matmuls overlap group k's bias-add drain), tag "o"
        # 2 bufs x 1 bank for the o accumulators = 8 banks total.
        ps_sc = ctx.enter_context(tc.tile_pool(name="ps_sc", bufs=2, space="PSUM"))
        ps_pp = ctx.enter_context(tc.tile_pool(name="ps_pp", bufs=2, space="PSUM"))
        ps_o = ctx.enter_context(tc.tile_pool(name="ps_o", bufs=2, space="PSUM"))

        # ---------------- load phase (consolidated DMAs) --------------------
        wq_sb = weights.tile([P, NDC, D], f16, tag="wq")
        wk_sb = weights.tile([P, NDC, D], f16, tag="wk")
        wv_sb = weights.tile([P, NDC, D], f16, tag="wv")
        qt_sb = raw.tile([P, NDC, ROWS], f16, tag="qt")
        bqk_sb = weights.tile([P, 2 * NPAIR], f32, tag="bqk")
        # DMA issue is ~1us of engine time per descriptor batch; spread the
        # load DMAs across four engines so issue itself doesn't serialize.
        nc.sync.dma_start(out=wk_sb, in_=wk_d)
        nc.gpsimd.dma_start(out=qt_sb, in_=qt_d)
        nc.scalar.dma_start(out=wq_sb, in_=wq_d)
        nc.scalar.dma_start(out=bqk_sb, in_=bqk_d)
        kt_slabs = []
        for g in range(S // 512):
            kt_slab = raw.tile([P, NDC, 512], f16, tag=f"kt{g}", name=f"kt_slab{g}")
            nc.sync.dma_start(out=kt_slab, in_=kt_d[g])
            kt_slabs.append(kt_slab)
        nc.gpsimd.dma_start(out=wv_sb, in_=wv_d)
        vt_q = []
        for q in range(NTT // 4):
            vq = raw.tile([P, 4, NDC, 128], f16, tag=f"vt{q}", name=f"vt_q{q}")
            nc.gpsimd.dma_start(out=vq, in_=vt_d[q])
            vt_q.append(vq)
        wo_sb = weights.tile([DV, H, D], f16, tag="wo")
        bob_sb = weights.tile([P, D], f32, tag="bob")
        nc.gpsimd.dma_start(out=wo_sb, in_=wo_d)
        nc.gpsimd.dma_start(out=bob_sb, in_=bob_d)
        ones64 = weights.tile([VW, DV], f16, tag="ones64")
        nc.vector.memset(ones64, 1.0)

        def vt_slab(t):
            return vt_q[t // 4][:, t % 4]

        # -------------- compute phase (optionally looped for bench) ---------
        loop_cm = (
            tc.For_i(
                0, hw_loop, 1, name="bench",
                hint_engines=(
                    mybir.EngineType.PE,
                    mybir.EngineType.Activation,
                    mybir.EngineType.DVE,
                    mybir.EngineType.SP,
                ),
            )
            if hw_loop
            else contextlib.nullcontext()
        )
        with loop_cm:
          for _rep in range(repeats):
            KT = [acts.tile([P, S], f16, tag=f"KT{p}", name=f"KT{p}") for p in range(NPAIR)]
            QT = [acts.tile([P, ROWS], f16, tag=f"QT{p}", name=f"QT{p}") for p in range(NPAIR)]
            Vp = [
                acts.tile([P, H, VW], f16, tag=f"Vp{t}", name=f"Vp{t}")
                for t in range(NTT)
            ]
            o2T = [acts.tile([DV, ROWS], f16, tag=f"o2T{i}", name=f"o2T{i}") for i in range(H)]
            den64 = acts.tile([VW, H, ROWS], f16, tag="den64", name="den64")

            def proj_qt(p):
                ps = ps_pp.tile([P, ROWS], f32, tag="pp", name="ps_q")
                for c in range(NDC):
                    nc.tensor.matmul(
                        ps, lhsT=wq_sb[:, c, p * 128 : (p + 1) * 128],
                        rhs=qt_sb[:, c, :],
                        start=(c == 0), stop=(c == NDC - 1),
                    )
                nc.vector.tensor_scalar_add(QT[p], ps, bqk_sb[:, p : p + 1])

            def proj_kt(p, g):
                ps = ps_pp.tile([P, 512], f32, tag="pp", name="ps_k")
                for c in range(NDC):
                    nc.tensor.matmul(
                        ps, lhsT=wk_sb[:, c, p * 128 : (p + 1) * 128],
                        rhs=kt_slabs[g][:, c, :],
                        start=(c == 0), stop=(c == NDC - 1),
                    )
                nc.vector.tensor_scalar_add(
                    KT[p][:, g * 512 : (g + 1) * 512], ps,
                    bqk_sb[:, NPAIR + p : NPAIR + p + 1],
                )

            def proj_v(t):
                ps = ps_pp.tile([P, D], f32, tag="pp", name="ps_v")
                for c in range(NDC):
                    nc.tensor.matmul(
                        ps, lhsT=vt_slab(t)[:, c, :], rhs=wv_sb[:, c, :],
                        start=(c == 0), stop=(c == NDC - 1),
                    )
                nc.vector.memset(Vp[t][:, :, DV : DV + 1], 1.0)
                nc.vector.tensor_copy(
                    Vp[t][:, :, 0:DV], ps.rearrange("p (i v) -> p i v", i=H)
                )

            def scores1(p, t):
                # scores for pair p, key tile t; one N=1024 exp (2 banks)
                ps = ps_sc.tile([P, 2, 512], f32, tag="sc", name="ps_sc_t")
                ts = slice(t * 128, (t + 1) * 128)
                for i in range(2):
                    nc.tensor.matmul(
                        ps[:, i, :],
                        lhsT=KT[p][64 * i : 64 * i + 64, ts],
                        rhs=QT[p][64 * i : 64 * i + 64, :],
                        start=True, stop=True,
                    )
                at = attn_pool.tile([P, 2, 512], f16, tag="at", name="at_t")
                nc.scalar.activation(at, ps, Exp, scale=1.0 / np.sqrt(DK))
                return at

            def ov_step1(p, o_ps, at, t):
                for i in range(2):
                    nc.tensor.matmul(
                        o_ps[i], lhsT=Vp[t][:, 2 * p + i, :], rhs=at[:, i, :],
                        start=(t == 0), stop=(t == NTT - 1),
                    )

            def den_copy(p, o_ps):
                for i in range(2):
                    nc.vector.tensor_copy(
                        den64[DV : DV + 1, 2 * p + i, :], o_ps[i][DV : DV + 1, :]
                    )

            def ov_finish(p, o_ps):
                # rows 0:64 = unnormalized head output, row 64 = softmax denom
                for i in range(2):
                    bc = ps_pp.tile([DV, 512], f32, tag="pp", name="bc_ps")
                    nc.tensor.matmul(
                        bc, lhsT=ones64[DV : DV + 1, :],
                        rhs=den64[DV : DV + 1, 2 * p + i, :], start=True, stop=True,
                    )
                    rbc = small.tile([DV, 512], f32, tag="rbc", name="rbc")
                    nc.vector.reciprocal_approx_fast(rbc, bc)
                    nc.vector.tensor_mul(o2T[2 * p + i], o_ps[i][0:DV, :], rbc)

            # Projection work interleaved into the attention rounds, keyed by
            # (pair, t). KT[p]/QT[p] must complete before pair p's scores.
            interleave = {
                (0, 0): [("v", 2)], (0, 1): [("v", 3)],
                (0, 2): [("v", 4), ("kt", 1, 0)], (0, 3): [("v", 5)],
                (0, 4): [("v", 6)], (0, 5): [("v", 7), ("kt", 1, 1)],
                (0, 6): [("v", 8)], (0, 7): [("v", 9)],
                (0, 8): [("v", 10), ("kt", 1, 2)], (0, 9): [("v", 11)],
                (0, 10): [("v", 12)], (0, 11): [("v", 13), ("kt", 1, 3)],
                (0, 12): [("v", 14)], (0, 13): [("v", 15)],
                (0, 14): [("qt", 1)], (0, 15): [("kt", 2, 0)],
                (1, 1): [("kt", 2, 1)], (1, 3): [("kt", 2, 2)],
                (1, 5): [("kt", 2, 3)], (1, 7): [("kt", 3, 0)],
                (1, 9): [("kt", 3, 1)], (1, 11): [("qt", 2)],
                (1, 13): [("kt", 3, 2)], (1, 15): [("kt", 3, 3)],
                (2, 1): [("qt", 3)],
            }

            def do_interleave(p, t):
                for item in interleave.get((p, t), []):
                    if item[0] == "v":
                        proj_v(item[1])
                    elif item[0] == "kt":
                        proj_kt(item[1], item[2])
                    else:
                        proj_qt(item[1])

            # --- phase A: prologue for pair 0 + two V lead tiles (more V here
            # --- would stall the FIFO on the vt DMAs; the rest stream in
            # --- just-in-time via the interleave)
            proj_qt(0)
            for g in range(S // 512):
                proj_kt(0, g)
            proj_v(0)
            proj_v(1)

            # --- attention rounds, o-matmuls lag scores by one round so they
            # --- never stall the PE FIFO on the exp. Pair normalization is
            # --- issued one round after the pair's last o accumulation and
            # --- MUST precede the next pair's second o-step (its o-bank reuse
            # --- waits on norm work that would otherwise sit later in the PE
            # --- FIFO behind the stalled matmul).
            rounds = [(p, t) for p in range(NPAIR) for t in range(NTT)]
            o_ps_by_pair = {}
            prev = None
            held = None
            for p, t in rounds:
                if t == 0:
                    o_ps_by_pair[p] = [
                        ps_o.tile([VW, ROWS], f32, tag="o", name=f"o_ps{i}")
                        for i in range(2)
                    ]
                at = scores1(p, t)
                if prev is not None:
                    pp, pt, pat = prev
                    if pt == 0:
                        # hold the pair's first o-step one extra round so the
                        # o-bank reuse never stalls the FIFO on the previous
                        # pair's normalization chain
                        held = prev
                    else:
                        if held is not None:
                            hp, ht, hat = held
                            ov_step1(hp, o_ps_by_pair[hp], hat, ht)
                            held = None
                        ov_step1(pp, o_ps_by_pair[pp], pat, pt)
                        if pt == NTT - 1:
                            den_copy(pp, o_ps_by_pair[pp])
                            ov_finish(pp, o_ps_by_pair[pp])
                prev = (p, t, at)
                do_interleave(p, t)
            pp, pt, pat = prev
            ov_step1(pp, o_ps_by_pair[pp], pat, pt)
            den_copy(pp, o_ps_by_pair[pp])
            ov_finish(pp, o_ps_by_pair[pp])

            # --- output projection for this core's 512 rows
            for st in range(ROWS // P):
                ps = ps_pp.tile([P, D], f32, tag="pp", name="ps_out")
                for i in range(H):
                    nc.tensor.matmul(
                        ps, lhsT=o2T[i][:, st * 128 : (st + 1) * 128],
                        rhs=wo_sb[:, i, :],
                        start=(i == 0), stop=(i == H - 1),
                    )
                ot = small.tile([P, D], f32, tag="ot")
                nc.vector.tensor_add(ot, ps, bob_sb)
                nc.gpsimd.dma_start(out=out_d[st], in_=ot)

    nc.compile()
    return nc


def _get_program(repeats=1, hw_loop=0):
    key = (repeats, hw_loop)
    if key not in _prog:
        _prog[key] = _build_program(repeats=repeats, hw_loop=hw_loop)
    return _prog[key]


def _stage_inputs(queries, keys, values, wq, bq, wk, bk, wv, bv, wo, bo):
    """Host staging: transpose activations to [D, S], chunk weights, slice
    per-core query shards. Returns the 8 per-core input dicts."""
    h = np.float16
    qT = queries.transpose(0, 2, 1).astype(h)
    kT = keys.transpose(0, 2, 1).astype(h)
    vT = values.transpose(0, 2, 1).astype(h)

    def chunk(m):
        # [512, X] -> [128, NDC, X]: row c*128+p -> [p, c, :]
        return np.ascontiguousarray(m.reshape(NDC, P, m.shape[1]).transpose(1, 0, 2))

    wq_m = chunk(np.concatenate([wq[i] for i in range(H)], axis=1)).astype(h)
    wk_m = chunk(np.concatenate([wk[i] for i in range(H)], axis=1)).astype(h)
    wv_m = chunk(np.concatenate([wv[i] for i in range(H)], axis=1)).astype(h)
    wo_m = np.ascontiguousarray(wo.reshape(H, DV, D).transpose(1, 0, 2)).astype(h)
    bqk = np.concatenate(
        [bq.reshape(NPAIR, P).T, bk.reshape(NPAIR, P).T], axis=1
    ).astype(np.float32)
    bqk = np.ascontiguousarray(bqk)
    # fold bv through the output projection: out += concat(bv) @ wo
    bo_eff = (bo + bv.reshape(D) @ wo).astype(np.float32)
    bob = np.broadcast_to(bo_eff.reshape(1, D), (P, D)).astype(np.float32).copy()

    # kt slab layout [g, p, c, x]: kt[g,p,c,x] = kT[b][c*128+p, g*512+x]
    kt_b = [
        np.ascontiguousarray(kT[b].reshape(NDC, P, S // 512, 512).transpose(2, 1, 0, 3))
        for b in range(B)
    ]
    # vt layout [q, p, u, c, x]: tile t=4q+u; vt[...] = vT[b][c*128+p, t*128+x]
    vt_b = [
        np.ascontiguousarray(
            vT[b].reshape(NDC, P, NTT // 4, 4, 128).transpose(2, 1, 3, 0, 4)
        )
        for b in range(B)
    ]
    in_maps = []
    for c in range(NCORES):
        b, r = c // 4, c % 4
        qt_c = np.ascontiguousarray(
            qT[b][:, r * ROWS : (r + 1) * ROWS].reshape(NDC, P, ROWS).transpose(1, 0, 2)
        )
        in_maps.append(
            {
                "qt": qt_c,
                "kt": kt_b[b],
                "vt": vt_b[b],
                "wq": wq_m, "wk": wk_m, "wv": wv_m, "wo": wo_m,
                "bqk": bqk, "bob": bob,
            }
        )
    return in_maps


def run(trace=False, repeats=1, hw_loop=0, **inputs):
    """Run the kernel; returns (output, BassKernelResults)."""
    from concourse.bass_utils import run_bass_kernel_spmd

    nc = _get_program(repeats, hw_loop)
    in_maps = _stage_inputs(**inputs)
    res = run_bass_kernel_spmd(nc, in_maps, core_ids=list(range(NCORES)), trace=trace)
    out = np.empty((B, S, D), np.float32)
    for c in range(NCORES):
        b, r = c // 4, c % 4
        out[b, r * ROWS : (r + 1) * ROWS, :] = res.results[c]["out"].reshape(ROWS, D)
    return out, res


def kernel(**inputs):
    out, _ = run(trace=False, **inputs)
    return out


# revision 26
# speedup vs baseline: 1.0060x; 1.0060x over previous
"""Multi-head attention kernel for 8 Trainium2 NeuronCores.

Problem: B=2, S=2048, H=8, DK=DV=64, D=512 (nn_MultiHeadAttention).

Sharding: core c owns batch b=c//4 and query rows [512*r, 512*r+512) with
r = c%4. No collectives: each core recomputes the full K/V projections for
its batch locally (the ~25us of redundant PE work is far cheaper than the
barrier + AllGather latency it replaces).

Per-core device kernel, software-pipelined so the PE FIFO never waits on
the exp: round k issues scores(k) then the o-matmuls of round k-1, so each
o only executes after its attention tile finished exp ~1 round earlier.
  QT[p]   = wq2[p].T @ qT + bq              [128, 512]
  KT[p]   = wk2[p].T @ kT + bk              [128, 2048]
  V'[t]   = vT(t).T @ wv | ones col         [128, 8, 65]  (65th column of
            ones makes the o-matmul emit the softmax denominator in row 64)
  scoresT = KT[p] halves @ QT[p]            [128, 2, 512] per (pair, t),
                                            2 concurrent row-group matmuls
  attnT   = exp(scoresT / 8)                ScalarE, f16, no max-subtract
  o65[h] += V'[t,h] @ attnT[:, i]           accumulated over t; row 64 =
                                            softmax denominator
  bc      = ones(1x64).T @ denom row        K=1 matmul partition-broadcast
  rbc     = reciprocal_approx_fast(bc)      one DVE op per pair
  o2T[h]  = o65[h][0:64] * rbc              DVE, f16
  out     = sum_h o2T[h].T-slices @ wo[h] + bo'
bv is folded into the output bias on the host (bo' = bo + concat(bv) @ wo),
so the V projection needs no bias add on device.
"""

import numpy as np

B, S, H, DK, DV = 2, 2048, 8, 64, 64
D = H * DV  # 512
NCORES = 8
GROUP = 4  # cores per batch
ROWS = (B * S) // NCORES  # 512 query rows per core
NPAIR = H // 2  # 4 head pairs
NTT = S // 128  # 16 key/value tiles
NDC = D // 128  # 4 contraction chunks
P = 128
VW = DV + 1  # 65: V columns per head incl. the ones column

_prog = {}


def _build_program(repeats=1, hw_loop=0):
    from contextlib import ExitStack
    import contextlib

    import concourse.mybir as mybir
    import concourse.tile as tile
    from concourse import bacc

    f32 = mybir.dt.float32
    f16 = mybir.dt.float16  # fp16 PE datapath: separate+fast weight loads
    Exp = mybir.ActivationFunctionType.Exp

    nc = bacc.Bacc("TRN2", target_bir_lowering=False, debug=False, num_devices=NCORES)

    # DRAM I/O (per-core data; same program on all 8 cores)
    qt_d = nc.dram_tensor("qt", [P, NDC, ROWS], f16, kind="ExternalInput").ap()
    kt_d = nc.dram_tensor("kt", [S // 512, P, NDC, 512], f16, kind="ExternalInput").ap()
    vt_d = nc.dram_tensor("vt", [NTT // 4, P, 4, NDC, 128], f16, kind="ExternalInput").ap()
    wq_d = nc.dram_tensor("wq", [P, NDC, D], f16, kind="ExternalInput").ap()
    wk_d = nc.dram_tensor("wk", [P, NDC, D], f16, kind="ExternalInput").ap()
    wv_d = nc.dram_tensor("wv", [P, NDC, D], f16, kind="ExternalInput").ap()
    wo_d = nc.dram_tensor("wo", [DV, H, D], f16, kind="ExternalInput").ap()
    bqk_d = nc.dram_tensor("bqk", [P, 2 * NPAIR], f32, kind="ExternalInput").ap()
    bob_d = nc.dram_tensor("bob", [P, D], f32, kind="ExternalInput").ap()
    out_d = nc.dram_tensor("out", [ROWS // P, P, D], f32, kind="ExternalOutput").ap()

    with tile.TileContext(nc) as tc, ExitStack() as ctx:
        weights = ctx.enter_context(tc.tile_pool(name="weights", bufs=1))
        raw = ctx.enter_context(tc.tile_pool(name="raw", bufs=1))
        acts = ctx.enter_context(tc.tile_pool(name="acts", bufs=1))
        attn_pool = ctx.enter_context(tc.tile_pool(name="attn", bufs=4))
        small = ctx.enter_context(tc.tile_pool(name="small", bufs=2))
        # PSUM: tag "sc" 2 bufs x [128,2,512]f32 (2 banks each) dedicated to
        # scores (strict double-buffer against the exp), tag "pp" 2 bufs x
        # 1 bank for projection groups + denom broadcasts (double-buffered so
        # proj group k+1 matmuls overlap group k's bias-add drain), tag "o"
        # 2 bufs x 1 bank for the o accumulators = 8 banks total.
        ps_sc = ctx.enter_context(tc.tile_pool(name="ps_sc", bufs=2, space="PSUM"))
        ps_pp = ctx.enter_context(tc.tile_pool(name="ps_pp", bufs=2, space="PSUM"))
        ps_o = ctx.enter_context(tc.tile_pool(name="ps_o", bufs=2, space="PSUM"))

        # ---------------- load phase (consolidated DMAs) --------------------
        wq_sb = weights.tile([P, NDC, D], f16, tag="wq")
        wk_sb = weights.tile([P, NDC, D], f16, tag="wk")
        wv_sb = weights.tile([P, NDC, D], f16, tag="wv")
        qt_sb = raw.tile([P, NDC, ROWS], f16, tag="qt")
        bqk_sb = weights.tile([P, 2 * NPAIR], f32, tag="bqk")
        # DMA issue is ~1us of engine time per descriptor batch; spread the
        # load DMAs across four engines so issue itself doesn't serialize.
        nc.sync.dma_start(out=wk_sb, in_=wk_d)
        nc.gpsimd.dma_start(out=qt_sb, in_=qt_d)
        nc.scalar.dma_start(out=wq_sb, in_=wq_d)
        nc.scalar.dma_start(out=bqk_sb, in_=bqk_d)
        kt_slabs = []
        for g in range(S // 512):
            kt_slab = raw.tile([P, NDC, 512], f16, tag=f"kt{g}", name=f"kt_slab{g}")
            nc.sync.dma_start(out=kt_slab, in_=kt_d[g])
            kt_slabs.append(kt_slab)
        nc.gpsimd.dma_start(out=wv_sb, in_=wv_d)
        vt_q = []
        for q in range(NTT // 4):
            vq = raw.tile([P, 4, NDC, 128], f16, tag=f"vt{q}", name=f"vt_q{q}")
            nc.gpsimd.dma_start(out=vq, in_=vt_d[q])
            vt_q.append(vq)
        wo_sb = weights.tile([DV, H, D], f16, tag="wo")
        bob_sb = weights.tile([P, D], f32, tag="bob")
        nc.gpsimd.dma_start(out=wo_sb, in_=wo_d)
        nc.gpsimd.dma_start(out=bob_sb, in_=bob_d)
        ones64 = weights.tile([VW, DV], f16, tag="ones64")
        nc.vector.memset(ones64, 1.0)

        def vt_slab(t):
            return vt_q[t // 4][:, t % 4]

        # -------------- compute phase (optionally looped for bench) ---------
        loop_cm = (
            tc.For_i(
                0, hw_loop, 1, name="bench",
                hint_engines=(
                    mybir.EngineType.PE,
                    mybir.EngineType.Activation,
                    mybir.EngineType.DVE,
                    mybir.EngineType.SP,
                ),
            )
            if hw_loop
            else contextlib.nullcontext()
        )
        with loop_cm:
          for _rep in range(repeats):
            KT = [acts.tile([P, S], f16, tag=f"KT{p}", name=f"KT{p}") for p in range(NPAIR)]
            QT = [acts.tile([P, ROWS], f16, tag=f"QT{p}", name=f"QT{p}") for p in range(NPAIR)]
            Vp = [
                acts.tile([P, H, VW], f16, tag=f"Vp{t}", name=f"Vp{t}")
                for t in range(NTT)
            ]
            o2T = [acts.tile([DV, ROWS], f16, tag=f"o2T{i}", name=f"o2T{i}") for i in range(H)]
            den64 = acts.tile([VW, H, ROWS], f16, tag="den64", name="den64")

            def proj_qt(p):
                ps = ps_pp.tile([P, ROWS], f32, tag="pp", name="ps_q")
                for c in range(NDC):
                    nc.tensor.matmul(
                        ps, lhsT=wq_sb[:, c, p * 128 : (p + 1) * 128],
                        rhs=qt_sb[:, c, :],
                        start=(c == 0), stop=(c == NDC - 1),
                    )
                nc.vector.tensor_scalar_add(QT[p], ps, bqk_sb[:, p : p + 1])

            def proj_kt(p, g):
                ps = ps_pp.tile([P, 512], f32, tag="pp", name="ps_k")
                for c in range(NDC):
                    nc.tensor.matmul(
                        ps, lhsT=wk_sb[:, c, p * 128 : (p + 1) * 128],
                        rhs=kt_slabs[g][:, c, :],
                        start=(c == 0), stop=(c == NDC - 1),
                    )
                nc.vector.tensor_scalar_add(
                    KT[p][:, g * 512 : (g + 1) * 512], ps,
                    bqk_sb[:, NPAIR + p : NPAIR + p + 1],
                )

            def proj_v(t):
                ps = ps_pp.tile([P, D], f32, tag="pp", name="ps_v")
                for c in range(NDC):
                    nc.tensor.matmul(
                        ps, lhsT=vt_slab(t)[:, c, :], rhs=wv_sb[:, c, :],
                        start=(c == 0), stop=(c == NDC - 1),
                    )
                nc.vector.memset(Vp[t][:, :, DV : DV + 1], 1.0)
                nc.vector.tensor_copy(
                    Vp[t][:, :, 0:DV], ps.rearrange("p (i v) -> p i v", i=H)
                )

            def scores1(p, t):
                # scores for pair p, key tile t; one N=1024 exp (2 banks)
                ps = ps_sc.tile([P, 2, 512], f32, tag="sc", name="ps_sc_t")
                ts = slice(t * 128, (t + 1) * 128)
                for i in range(2):
                    nc.tensor.matmul(
                        ps[:, i, :],
                        lhsT=KT[p][64 * i : 64 * i + 64, ts],
                        rhs=QT[p][64 * i : 64 * i + 64, :],
                        start=True, stop=True,
                    )
                at = attn_pool.tile([P, 2, 512], f16, tag="at", name="at_t")
                nc.scalar.activation(at, ps, Exp, scale=1.0 / np.sqrt(DK))
                return at

            def ov_step1(p, o_ps, at, t):
                for i in range(2):
                    nc.tensor.matmul(
                        o_ps[i], lhsT=Vp[t][:, 2 * p + i, :], rhs=at[:, i, :],
                        start=(t == 0), stop=(t == NTT - 1),
                    )

            def den_copy(p, o_ps):
                for i in range(2):
                    nc.vector.tensor_copy(
                        den64[DV : DV + 1, 2 * p + i, :], o_ps[i][DV : DV + 1, :]
                    )

            def ov_finish(p, o_ps):
                # rows 0:64 = unnormalized head output, row 64 = softmax denom
                for i in range(2):
                    bc = ps_pp.tile([DV, 512], f32, tag="pp", name="bc_ps")
                    nc.tensor.matmul(
                        bc, lhsT=ones64[DV : DV + 1, :],
                        rhs=den64[DV : DV + 1, 2 * p + i, :], start=True, stop=True,
                    )
                    rbc = small.tile([DV, 512], f32, tag="rbc", name="rbc")
                    nc.vector.reciprocal_approx_fast(rbc, bc)
                    nc.vector.tensor_mul(o2T[2 * p + i], o_ps[i][0:DV, :], rbc)

            # Projection work interleaved into the attention rounds, keyed by
            # (pair, t). KT[p]/QT[p] must complete before pair p's scores.
            interleave = {
                (0, 0): [("v", 2)], (0, 1): [("v", 3), ("kt", 0, 2)],
                (0, 2): [("v", 4)], (0, 3): [("v", 5)],
                (0, 4): [("v", 6), ("kt", 0, 3)], (0, 5): [("v", 7)],
                (0, 6): [("v", 8), ("kt", 1, 0)], (0, 7): [("v", 9)],
                (0, 8): [("v", 10), ("kt", 1, 1)], (0, 9): [("v", 11)],
                (0, 10): [("v", 12), ("kt", 1, 2)], (0, 11): [("v", 13)],
                (0, 12): [("v", 14), ("kt", 1, 3)], (0, 13): [("v", 15)],
                (0, 14): [("qt", 1)], (0, 15): [("kt", 2, 0)],
                (1, 1): [("kt", 2, 1)], (1, 3): [("kt", 2, 2)],
                (1, 5): [("kt", 2, 3)], (1, 7): [("kt", 3, 0)],
                (1, 9): [("kt", 3, 1)], (1, 11): [("qt", 2)],
                (1, 13): [("kt", 3, 2)], (1, 15): [("kt", 3, 3)],
                (2, 1): [("qt", 3)],
            }

            def do_interleave(p, t):
                for item in interleave.get((p, t), []):
                    if item[0] == "v":
                        proj_v(item[1])
                    elif item[0] == "kt":
                        proj_kt(item[1], item[2])
                    else:
                        proj_qt(item[1])

            # --- phase A: prologue for pair 0 + two V lead tiles (more V here
            # --- would stall the FIFO on the vt DMAs; the rest stream in
            # --- just-in-time via the interleave)
            proj_qt(0)
            proj_kt(0, 0)
            proj_kt(0, 1)
            proj_v(0)
            proj_v(1)

            # --- attention rounds, o-matmuls lag scores by one round so they
            # --- never stall the PE FIFO on the exp. Pair normalization is
            # --- issued one round after the pair's last o accumulation and
            # --- MUST precede the next pair's second o-step (its o-bank reuse
            # --- waits on norm work that would otherwise sit later in the PE
            # --- FIFO behind the stalled matmul).
            rounds = [(p, t) for p in range(NPAIR) for t in range(NTT)]
            o_ps_by_pair = {}
            prev = None
            held = None
            for p, t in rounds:
                if t == 0:
                    o_ps_by_pair[p] = [
                        ps_o.tile([VW, ROWS], f32, tag="o", name=f"o_ps{i}")
                        for i in range(2)
                    ]
                at = scores1(p, t)
                if prev is not None:
                    pp, pt, pat = prev
                    if pt == 0:
                        # hold the pair's first o-step one extra round so the
                        # o-bank reuse never stalls the FIFO on the previous
                        # pair's normalization chain
                        held = prev
                    else:
                        if held is not None:
                            hp, ht, hat = held
                            ov_step1(hp, o_ps_by_pair[hp], hat, ht)
                            held = None
                        ov_step1(pp, o_ps_by_pair[pp], pat, pt)
                        if pt == NTT - 1:
                            den_copy(pp, o_ps_by_pair[pp])
                            ov_finish(pp, o_ps_by_pair[pp])
                prev = (p, t, at)
                do_interleave(p, t)
            pp, pt, pat = prev
            ov_step1(pp, o_ps_by_pair[pp], pat, pt)
            den_copy(pp, o_ps_by_pair[pp])
            ov_finish(pp, o_ps_by_pair[pp])

            # --- output projection for this core's 512 rows
            for st in range(ROWS // P):
                ps = ps_pp.tile([P, D], f32, tag="pp", name="ps_out")
                for i in range(H):
                    nc.tensor.matmul(
                        ps, lhsT=o2T[i][:, st * 128 : (st + 1) * 128],
                        rhs=wo_sb[:, i, :],
                        start=(i == 0), stop=(i == H - 1),
                    )
                ot = small.tile([P, D], f32, tag="ot")
                nc.vector.tensor_add(ot, ps, bob_sb)
                nc.gpsimd.dma_start(out=out_d[st], in_=ot)

    nc.compile()
    return nc


def _get_program(repeats=1, hw_loop=0):
    key = (repeats, hw_loop)
    if key not in _prog:
        _prog[key] = _build_program(repeats=repeats, hw_loop=hw_loop)
    return _prog[key]


def _stage_inputs(queries, keys, values, wq, bq, wk, bk, wv, bv, wo, bo):
    """Host staging: transpose activations to [D, S], chunk weights, slice
    per-core query shards. Returns the 8 per-core input dicts."""
    h = np.float16
    qT = queries.transpose(0, 2, 1).astype(h)
    kT = keys.transpose(0, 2, 1).astype(h)
    vT = values.transpose(0, 2, 1).astype(h)

    def chunk(m):
        # [512, X] -> [128, NDC, X]: row c*128+p -> [p, c, :]
        return np.ascontiguousarray(m.reshape(NDC, P, m.shape[1]).transpose(1, 0, 2))

    wq_m = chunk(np.concatenate([wq[i] for i in range(H)], axis=1)).astype(h)
    wk_m = chunk(np.concatenate([wk[i] for i in range(H)], axis=1)).astype(h)
    wv_m = chunk(np.concatenate([wv[i] for i in range(H)], axis=1)).astype(h)
    wo_m = np.ascontiguousarray(wo.reshape(H, DV, D).transpose(1, 0, 2)).astype(h)
    bqk = np.concatenate(
        [bq.reshape(NPAIR, P).T, bk.reshape(NPAIR, P).T], axis=1
    ).astype(np.float32)
    bqk = np.ascontiguousarray(bqk)
    # fold bv through the output projection: out += concat(bv) @ wo
    bo_eff = (bo + bv.reshape(D) @ wo).astype(np.float32)
    bob = np.broadcast_to(bo_eff.reshape(1, D), (P, D)).astype(np.float32).copy()

    # kt slab layout [g, p, c, x]: kt[g,p,c,x] = kT[b][c*128+p, g*512+x]
    kt_b = [
        np.ascontiguousarray(kT[b].reshape(NDC, P, S // 512, 512).transpose(2, 1, 0, 3))
        for b in range(B)
    ]
    # vt layout [q, p, u, c, x]: tile t=4q+u; vt[...] = vT[b][c*128+p, t*128+x]
    vt_b = [
        np.ascontiguousarray(
            vT[b].reshape(NDC, P, NTT // 4, 4, 128).transpose(2, 1, 3, 0, 4)
        )
        for b in range(B)
    ]
    in_maps = []
    for c in range(NCORES):
        b, r = c // 4, c % 4
        qt_c = np.ascontiguousarray(
            qT[b][:, r * ROWS : (r + 1) * ROWS].reshape(NDC, P, ROWS).transpose(1, 0, 2)
        )
        in_maps.append(
            {
                "qt": qt_c,
                "kt": kt_b[b],
                "vt": vt_b[b],
                "wq": wq_m, "wk": wk_m, "wv": wv_m, "wo": wo_m,
                "bqk": bqk, "bob": bob,
            }
        )
    return in_maps


def run(trace=False, repeats=1, hw_loop=0, **inputs):
    """Run the kernel; returns (output, BassKernelResults)."""
    from concourse.bass_utils import run_bass_kernel_spmd

    nc = _get_program(repeats, hw_loop)
    in_maps = _stage_inputs(**inputs)
    res = run_bass_kernel_spmd(nc, in_maps, core_ids=list(range(NCORES)), trace=trace)
    out = np.empty((B, S, D), np.float32)
    for c in range(NCORES):
        b, r = c // 4, c % 4
        out[b, r * ROWS : (r + 1) * ROWS, :] = res.results[c]["out"].reshape(ROWS, D)
    return out, res


def kernel(**inputs):
    out, _ = run(trace=False, **inputs)
    return out


# revision 27
# speedup vs baseline: 1.0193x; 1.0133x over previous
"""Multi-head attention kernel for 8 Trainium2 NeuronCores.

Problem: B=2, S=2048, H=8, DK=DV=64, D=512 (nn_MultiHeadAttention).

Sharding: core c owns batch b=c//4 and query rows [512*r, 512*r+512) with
r = c%4. No collectives: each core recomputes the full K/V projections for
its batch locally (the ~25us of redundant PE work is far cheaper than the
barrier + AllGather latency it replaces).

Per-core device kernel, software-pipelined so the PE FIFO never waits on
the exp: round k issues scores(k) then the o-matmuls of round k-1, so each
o only executes after its attention tile finished exp ~1 round earlier.
  QT[p]   = wq2[p].T @ qT + bq              [128, 512]
  KT[p]   = wk2[p].T @ kT + bk              [128, 2048]
  V'[t]   = vT(t).T @ wv | ones col         [128, 8, 65]  (65th column of
            ones makes the o-matmul emit the softmax denominator in row 64)
  scoresT = KT[p] halves @ QT[p]            [128, 2, 512] per (pair, t),
                                            2 concurrent row-group matmuls
  attnT   = exp(scoresT / 8)                ScalarE, f16, no max-subtract
  o65[h] += V'[t,h] @ attnT[:, i]           accumulated over t; row 64 =
                                            softmax denominator
  bc      = ones(1x64).T @ denom row        K=1 matmul partition-broadcast
  rbc     = reciprocal_approx_fast(bc)      one DVE op per pair
  o2T[h]  = o65[h][0:64] * rbc              DVE, f16
  out     = sum_h o2T[h].T-slices @ wo[h] + bo'
bv is folded into the output bias on the host (bo' = bo + concat(bv) @ wo),
so the V projection needs no bias add on device.
"""

import numpy as np

B, S, H, DK, DV = 2, 2048, 8, 64, 64
D = H * DV  # 512
NCORES = 8
GROUP = 4  # cores per batch
ROWS = (B * S) // NCORES  # 512 query rows per core
NPAIR = H // 2  # 4 head pairs
NTT = S // 128  # 16 key/value tiles
NDC = D // 128  # 4 contraction chunks
P = 128
VW = DV + 1  # 65: V columns per head incl. the ones column

_prog = {}


def _build_program(repeats=1, hw_loop=0):
    from contextlib import ExitStack
    import contextlib

    import concourse.mybir as mybir
    import concourse.tile as tile
    from concourse import bacc

    f32 = mybir.dt.float32
    f16 = mybir.dt.float16  # fp16 PE datapath: separate+fast weight loads
    Exp = mybir.ActivationFunctionType.Exp

    nc = bacc.Bacc("TRN2", target_bir_lowering=False, debug=False, num_devices=NCORES)

    # DRAM I/O (per-core data; same program on all 8 cores)
    qt_d = nc.dram_tensor("qt", [P, NDC, ROWS], f16, kind="ExternalInput").ap()
    kt_d = nc.dram_tensor("kt", [S // 512, P, NDC, 512], f16, kind="ExternalInput").ap()
    vt_d = nc.dram_tensor("vt", [NTT // 4, P, 4, NDC, 128], f16, kind="ExternalInput").ap()
    wq_d = nc.dram_tensor("wq", [P, NDC, D], f16, kind="ExternalInput").ap()
    wk_d = nc.dram_tensor("wk", [P, NDC, D], f16, kind="ExternalInput").ap()
    wv_d = nc.dram_tensor("wv", [P, NDC, D], f16, kind="ExternalInput").ap()
    wo_d = nc.dram_tensor("wo", [DV, H, D], f16, kind="ExternalInput").ap()
    bqk_d = nc.dram_tensor("bqk", [P, 2 * NPAIR], f32, kind="ExternalInput").ap()
    bob_d = nc.dram_tensor("bob", [P, D], f32, kind="ExternalInput").ap()
    out_d = nc.dram_tensor("out", [ROWS // P, P, D], f32, kind="ExternalOutput").ap()

    with tile.TileContext(nc) as tc, ExitStack() as ctx:
        weights = ctx.enter_context(tc.tile_pool(name="weights", bufs=1))
        raw = ctx.enter_context(tc.tile_pool(name="raw", bufs=1))
        acts = ctx.enter_context(tc.tile_pool(name="acts", bufs=1))
        attn_pool = ctx.enter_context(tc.tile_pool(name="attn", bufs=4))
        small = ctx.enter_context(tc.tile_pool(name="small", bufs=2))
        # PSUM: tag "sc" 2 bufs x [128,2,512]f32 (2 banks each) dedicated to
        # scores (strict double-buffer against the exp), tag "pp" 2 bufs x
        # 1 bank for projection groups + denom broadcasts (double-buffered so
        # proj group k+1 matmuls overlap group k's bias-add drain), tag "o"
        # 2 bufs x 1 bank for the o accumulators = 8 banks total.
        ps_sc = ctx.enter_context(tc.tile_pool(name="ps_sc", bufs=2, space="PSUM"))
        ps_pp = ctx.enter_context(tc.tile_pool(name="ps_pp", bufs=2, space="PSUM"))
        ps_o = ctx.enter_context(tc.tile_pool(name="ps_o", bufs=2, space="PSUM"))

        # ---------------- load phase (consolidated DMAs) --------------------
        wq_sb = weights.tile([P, NDC, D], f16, tag="wq")
        wk_sb = weights.tile([P, NDC, D], f16, tag="wk")
        wv_sb = weights.tile([P, NDC, D], f16, tag="wv")
        qt_sb = raw.tile([P, NDC, ROWS], f16, tag="qt")
        bqk_sb = weights.tile([P, 2 * NPAIR], f32, tag="bqk")
        # DMA issue is ~1us of engine time per descriptor batch; spread the
        # load DMAs across four engines so issue itself doesn't serialize.
        nc.sync.dma_start(out=wk_sb, in_=wk_d)
        nc.gpsimd.dma_start(out=qt_sb, in_=qt_d)
        nc.scalar.dma_start(out=wq_sb, in_=wq_d)
        nc.scalar.dma_start(out=bqk_sb, in_=bqk_d)
        kt_slabs = []
        for g in range(S // 512):
            kt_slab = raw.tile([P, NDC, 512], f16, tag=f"kt{g}", name=f"kt_slab{g}")
            nc.sync.dma_start(out=kt_slab, in_=kt_d[g])
            kt_slabs.append(kt_slab)
        nc.gpsimd.dma_start(out=wv_sb, in_=wv_d)
        vt_q = []
        for q in range(NTT // 4):
            vq = raw.tile([P, 4, NDC, 128], f16, tag=f"vt{q}", name=f"vt_q{q}")
            nc.gpsimd.dma_start(out=vq, in_=vt_d[q])
            vt_q.append(vq)
        wo_sb = weights.tile([DV, H, D], f16, tag="wo")
        bob_sb = weights.tile([P, D], f32, tag="bob")
        nc.gpsimd.dma_start(out=wo_sb, in_=wo_d)
        nc.gpsimd.dma_start(out=bob_sb, in_=bob_d)
        ones64 = weights.tile([VW, DV], f16, tag="ones64")
        nc.vector.memset(ones64, 1.0)

        def vt_slab(t):
            return vt_q[t // 4][:, t % 4]

        # -------------- compute phase (optionally looped for bench) ---------
        loop_cm = (
            tc.For_i(
                0, hw_loop, 1, name="bench",
                hint_engines=(
                    mybir.EngineType.PE,
                    mybir.EngineType.Activation,
                    mybir.EngineType.DVE,
                    mybir.EngineType.SP,
                ),
            )
            if hw_loop
            else contextlib.nullcontext()
        )
        with loop_cm:
          for _rep in range(repeats):
            KT = [acts.tile([P, S], f16, tag=f"KT{p}", name=f"KT{p}") for p in range(NPAIR)]
            QT = [acts.tile([P, ROWS], f16, tag=f"QT{p}", name=f"QT{p}") for p in range(NPAIR)]
            Vp = [
                acts.tile([P, H, VW], f16, tag=f"Vp{t}", name=f"Vp{t}")
                for t in range(NTT)
            ]
            o2T = [acts.tile([DV, ROWS], f16, tag=f"o2T{i}", name=f"o2T{i}") for i in range(H)]
            den64 = acts.tile([VW, H, ROWS], f16, tag="den64", name="den64")

            def proj_qt(p):
                ps = ps_pp.tile([P, ROWS], f32, tag="pp", name="ps_q")
                for c in range(NDC):
                    nc.tensor.matmul(
                        ps, lhsT=wq_sb[:, c, p * 128 : (p + 1) * 128],
                        rhs=qt_sb[:, c, :],
                        start=(c == 0), stop=(c == NDC - 1),
                    )
                nc.vector.tensor_scalar_add(QT[p], ps, bqk_sb[:, p : p + 1])

            def proj_kt(p, g):
                ps = ps_pp.tile([P, 512], f32, tag="pp", name="ps_k")
                for c in range(NDC):
                    nc.tensor.matmul(
                        ps, lhsT=wk_sb[:, c, p * 128 : (p + 1) * 128],
                        rhs=kt_slabs[g][:, c, :],
                        start=(c == 0), stop=(c == NDC - 1),
                    )
                nc.vector.tensor_scalar_add(
                    KT[p][:, g * 512 : (g + 1) * 512], ps,
                    bqk_sb[:, NPAIR + p : NPAIR + p + 1],
                )

            def proj_v(t):
                ps = ps_pp.tile([P, D], f32, tag="pp", name="ps_v")
                for c in range(NDC):
                    nc.tensor.matmul(
                        ps, lhsT=vt_slab(t)[:, c, :], rhs=wv_sb[:, c, :],
                        start=(c == 0), stop=(c == NDC - 1),
                    )
                nc.vector.memset(Vp[t][:, :, DV : DV + 1], 1.0)
                nc.vector.tensor_copy(
                    Vp[t][:, :, 0:DV], ps.rearrange("p (i v) -> p i v", i=H)
                )

            def scores1(p, t):
                # scores for pair p, key tile t; one N=1024 exp (2 banks)
                ps = ps_sc.tile([P, 2, 512], f32, tag="sc", name="ps_sc_t")
                ts = slice(t * 128, (t + 1) * 128)
                for i in range(2):
                    nc.tensor.matmul(
                        ps[:, i, :],
                        lhsT=KT[p][64 * i : 64 * i + 64, ts],
                        rhs=QT[p][64 * i : 64 * i + 64, :],
                        start=True, stop=True,
                    )
                at = attn_pool.tile([P, 2, 512], f16, tag="at", name="at_t")
                nc.scalar.activation(at, ps, Exp, scale=1.0 / np.sqrt(DK))
                return at

            def ov_step1(p, o_ps, at, t):
                for i in range(2):
                    nc.tensor.matmul(
                        o_ps[i], lhsT=Vp[t][:, 2 * p + i, :], rhs=at[:, i, :],
                        start=(t == 0), stop=(t == NTT - 1),
                    )

            def den_copy(p, o_ps):
                for i in range(2):
                    nc.vector.tensor_copy(
                        den64[DV : DV + 1, 2 * p + i, :], o_ps[i][DV : DV + 1, :]
                    )

            def ov_finish(p, o_ps):
                # rows 0:64 = unnormalized head output, row 64 = softmax denom
                for i in range(2):
                    bc = ps_pp.tile([DV, 512], f32, tag="pp", name="bc_ps")
                    nc.tensor.matmul(
                        bc, lhsT=ones64[DV : DV + 1, :],
                        rhs=den64[DV : DV + 1, 2 * p + i, :], start=True, stop=True,
                    )
                    rbc = small.tile([DV, 512], f32, tag="rbc", name="rbc")
                    nc.vector.reciprocal_approx_fast(rbc, bc)
                    nc.vector.tensor_mul(o2T[2 * p + i], o_ps[i][0:DV, :], rbc)

            # Projection work interleaved into the attention rounds, keyed by
            # (pair, t). KT[p]/QT[p] must complete before pair p's scores.
            interleave = {
                (0, 0): [("v", 2)], (0, 1): [("v", 3), ("kt", 0, 2)],
                (0, 2): [("v", 4)], (0, 3): [("v", 5)],
                (0, 4): [("v", 6), ("kt", 0, 3)], (0, 5): [("v", 7)],
                (0, 6): [("v", 8), ("kt", 1, 0)], (0, 7): [("v", 9)],
                (0, 8): [("v", 10), ("kt", 1, 1)], (0, 9): [("v", 11)],
                (0, 10): [("v", 12), ("kt", 1, 2)], (0, 11): [("v", 13)],
                (0, 12): [("v", 14), ("kt", 1, 3)], (0, 13): [("v", 15)],
                (0, 14): [("qt", 1)], (0, 15): [("kt", 2, 0)],
                (1, 1): [("kt", 2, 1)], (1, 3): [("kt", 2, 2)],
                (1, 5): [("kt", 2, 3)], (1, 7): [("kt", 3, 0)],
                (1, 9): [("kt", 3, 1)], (1, 11): [("qt", 2)],
                (1, 13): [("kt", 3, 2)], (1, 15): [("kt", 3, 3)],
                (2, 1): [("qt", 3)],
            }

            def do_interleave(p, t):
                for item in interleave.get((p, t), []):
                    if item[0] == "v":
                        proj_v(item[1])
                    elif item[0] == "kt":
                        proj_kt(item[1], item[2])
                    else:
                        proj_qt(item[1])

            # --- phase A: prologue for pair 0 + two V lead tiles (more V here
            # --- would stall the FIFO on the vt DMAs; the rest stream in
            # --- just-in-time via the interleave)
            proj_qt(0)
            proj_kt(0, 0)
            proj_kt(0, 1)
            proj_v(0)
            proj_v(1)

            # --- attention rounds, o-matmuls lag scores by one round so they
            # --- never stall the PE FIFO on the exp. Pair normalization is
            # --- issued one round after the pair's last o accumulation and
            # --- MUST precede the next pair's second o-step (its o-bank reuse
            # --- waits on norm work that would otherwise sit later in the PE
            # --- FIFO behind the stalled matmul).
            rounds = [(p, t) for p in range(NPAIR) for t in range(NTT)]
            o_ps_by_pair = {}
            pend = []
            for p, t in rounds:
                if t == 0:
                    o_ps_by_pair[p] = [
                        ps_o.tile([VW, ROWS], f32, tag="o", name=f"o_ps{i}")
                        for i in range(2)
                    ]
                at = scores1(p, t)
                pend.append((p, t, at))
                while len(pend) > 2:
                    # o-matmuls lag scores by two rounds so they never stall
                    # the in-order PE FIFO on an exp, even in rounds loaded
                    # with projection work
                    pp, pt, pat = pend.pop(0)
                    ov_step1(pp, o_ps_by_pair[pp], pat, pt)
                    if pt == NTT - 1:
                        den_copy(pp, o_ps_by_pair[pp])
                        ov_finish(pp, o_ps_by_pair[pp])
                do_interleave(p, t)
            while pend:
                pp, pt, pat = pend.pop(0)
                ov_step1(pp, o_ps_by_pair[pp], pat, pt)
                if pt == NTT - 1:
                    den_copy(pp, o_ps_by_pair[pp])
                    ov_finish(pp, o_ps_by_pair[pp])

            # --- output projection for this core's 512 rows
            for st in range(ROWS // P):
                ps = ps_pp.tile([P, D], f32, tag="pp", name="ps_out")
                for i in range(H):
                    nc.tensor.matmul(
                        ps, lhsT=o2T[i][:, st * 128 : (st + 1) * 128],
                        rhs=wo_sb[:, i, :],
                        start=(i == 0), stop=(i == H - 1),
                    )
                ot = small.tile([P, D], f32, tag="ot")
                nc.vector.tensor_add(ot, ps, bob_sb)
                nc.gpsimd.dma_start(out=out_d[st], in_=ot)

    nc.compile()
    return nc


def _get_program(repeats=1, hw_loop=0):
    key = (repeats, hw_loop)
    if key not in _prog:
        _prog[key] = _build_program(repeats=repeats, hw_loop=hw_loop)
    return _prog[key]


def _stage_inputs(queries, keys, values, wq, bq, wk, bk, wv, bv, wo, bo):
    """Host staging: transpose activations to [D, S], chunk weights, slice
    per-core query shards. Returns the 8 per-core input dicts."""
    h = np.float16
    qT = queries.transpose(0, 2, 1).astype(h)
    kT = keys.transpose(0, 2, 1).astype(h)
    vT = values.transpose(0, 2, 1).astype(h)

    def chunk(m):
        # [512, X] -> [128, NDC, X]: row c*128+p -> [p, c, :]
        return np.ascontiguousarray(m.reshape(NDC, P, m.shape[1]).transpose(1, 0, 2))

    wq_m = chunk(np.concatenate([wq[i] for i in range(H)], axis=1)).astype(h)
    wk_m = chunk(np.concatenate([wk[i] for i in range(H)], axis=1)).astype(h)
    wv_m = chunk(np.concatenate([wv[i] for i in range(H)], axis=1)).astype(h)
    wo_m = np.ascontiguousarray(wo.reshape(H, DV, D).transpose(1, 0, 2)).astype(h)
    bqk = np.concatenate(
        [bq.reshape(NPAIR, P).T, bk.reshape(NPAIR, P).T], axis=1
    ).astype(np.float32)
    bqk = np.ascontiguousarray(bqk)
    # fold bv through the output projection: out += concat(bv) @ wo
    bo_eff = (bo + bv.reshape(D) @ wo).astype(np.float32)
    bob = np.broadcast_to(bo_eff.reshape(1, D), (P, D)).astype(np.float32).copy()

    # kt slab layout [g, p, c, x]: kt[g,p,c,x] = kT[b][c*128+p, g*512+x]
    kt_b = [
        np.ascontiguousarray(kT[b].reshape(NDC, P, S // 512, 512).transpose(2, 1, 0, 3))
        for b in range(B)
    ]
    # vt layout [q, p, u, c, x]: tile t=4q+u; vt[...] = vT[b][c*128+p, t*128+x]
    vt_b = [
        np.ascontiguousarray(
            vT[b].reshape(NDC, P, NTT // 4, 4, 128).transpose(2, 1, 3, 0, 4)
        )
        for b in range(B)
    ]
    in_maps = []
    for c in range(NCORES):
        b, r = c // 4, c % 4
        qt_c = np.ascontiguousarray(
            qT[b][:, r * ROWS : (r + 1) * ROWS].reshape(NDC, P, ROWS).transpose(1, 0, 2)
        )
        in_maps.append(
            {
                "qt": qt_c,
                "kt": kt_b[b],
                "vt": vt_b[b],
                "wq": wq_m, "wk": wk_m, "wv": wv_m, "wo": wo_m,
                "bqk": bqk, "bob": bob,
            }
        )
    return in_maps


def run(trace=False, repeats=1, hw_loop=0, **inputs):
    """Run the kernel; returns (output, BassKernelResults)."""
    from concourse.bass_utils import run_bass_kernel_spmd

    nc = _get_program(repeats, hw_loop)
    in_maps = _stage_inputs(**inputs)
    res = run_bass_kernel_spmd(nc, in_maps, core_ids=list(range(NCORES)), trace=trace)
    out = np.empty((B, S, D), np.float32)
    for c in range(NCORES):
        b, r = c // 4, c % 4
        out[b, r * ROWS : (r + 1) * ROWS, :] = res.results[c]["out"].reshape(ROWS, D)
    return out, res


def kernel(**inputs):
    out, _ = run(trace=False, **inputs)
    return out


# revision 28
# speedup vs baseline: 1.0261x; 1.0066x over previous
"""Multi-head attention kernel for 8 Trainium2 NeuronCores.

Problem: B=2, S=2048, H=8, DK=DV=64, D=512 (nn_MultiHeadAttention).

Sharding: core c owns batch b=c//4 and query rows [512*r, 512*r+512) with
r = c%4. No collectives: each core recomputes the full K/V projections for
its batch locally (the ~25us of redundant PE work is far cheaper than the
barrier + AllGather latency it replaces).

Per-core device kernel, software-pipelined so the PE FIFO never waits on
the exp: round k issues scores(k) then the o-matmuls of round k-1, so each
o only executes after its attention tile finished exp ~1 round earlier.
  QT[p]   = wq2[p].T @ qT + bq              [128, 512]
  KT[p]   = wk2[p].T @ kT + bk              [128, 2048]
  V'[t]   = vT(t).T @ wv | ones col         [128, 8, 65]  (65th column of
            ones makes the o-matmul emit the softmax denominator in row 64)
  scoresT = KT[p] halves @ QT[p]            [128, 2, 512] per (pair, t),
                                            2 concurrent row-group matmuls
  attnT   = exp(scoresT / 8)                ScalarE, f16, no max-subtract
  o65[h] += V'[t,h] @ attnT[:, i]           accumulated over t; row 64 =
                                            softmax denominator
  bc      = ones(1x64).T @ denom row        K=1 matmul partition-broadcast
  rbc     = reciprocal_approx_fast(bc)      one DVE op per pair
  o2T[h]  = o65[h][0:64] * rbc              DVE, f16
  out     = sum_h o2T[h].T-slices @ wo[h] + bo'
bv is folded into the output bias on the host (bo' = bo + concat(bv) @ wo),
so the V projection needs no bias add on device.
"""

import numpy as np

B, S, H, DK, DV = 2, 2048, 8, 64, 64
D = H * DV  # 512
NCORES = 8
GROUP = 4  # cores per batch
ROWS = (B * S) // NCORES  # 512 query rows per core
NPAIR = H // 2  # 4 head pairs
NTT = S // 128  # 16 key/value tiles
NDC = D // 128  # 4 contraction chunks
P = 128
VW = DV + 1  # 65: V columns per head incl. the ones column

_prog = {}


def _build_program(repeats=1, hw_loop=0):
    from contextlib import ExitStack
    import contextlib

    import concourse.mybir as mybir
    import concourse.tile as tile
    from concourse import bacc

    f32 = mybir.dt.float32
    f16 = mybir.dt.float16  # fp16 PE datapath: separate+fast weight loads
    Exp = mybir.ActivationFunctionType.Exp

    nc = bacc.Bacc("TRN2", target_bir_lowering=False, debug=False, num_devices=NCORES)

    # DRAM I/O (per-core data; same program on all 8 cores)
    qt_d = nc.dram_tensor("qt", [P, NDC, ROWS], f16, kind="ExternalInput").ap()
    kt_d = nc.dram_tensor("kt", [S // 512, P, NDC, 512], f16, kind="ExternalInput").ap()
    vt_d = nc.dram_tensor("vt", [NTT // 4, P, 4, NDC, 128], f16, kind="ExternalInput").ap()
    wq_d = nc.dram_tensor("wq", [P, NDC, D], f16, kind="ExternalInput").ap()
    wk_d = nc.dram_tensor("wk", [P, NDC, D], f16, kind="ExternalInput").ap()
    wv_d = nc.dram_tensor("wv", [P, NDC, D], f16, kind="ExternalInput").ap()
    wo_d = nc.dram_tensor("wo", [DV, H, D], f16, kind="ExternalInput").ap()
    bqk_d = nc.dram_tensor("bqk", [P, 2 * NPAIR], f32, kind="ExternalInput").ap()
    bob_d = nc.dram_tensor("bob", [P, D], f32, kind="ExternalInput").ap()
    out_d = nc.dram_tensor("out", [ROWS // P, P, D], f32, kind="ExternalOutput").ap()

    with tile.TileContext(nc) as tc, ExitStack() as ctx:
        weights = ctx.enter_context(tc.tile_pool(name="weights", bufs=1))
        raw = ctx.enter_context(tc.tile_pool(name="raw", bufs=1))
        acts = ctx.enter_context(tc.tile_pool(name="acts", bufs=1))
        attn_pool = ctx.enter_context(tc.tile_pool(name="attn", bufs=4))
        small = ctx.enter_context(tc.tile_pool(name="small", bufs=2))
        # PSUM: tag "sc" 2 bufs x [128,2,512]f32 (2 banks each) dedicated to
        # scores (strict double-buffer against the exp), tag "pp" 2 bufs x
        # 1 bank for projection groups + denom broadcasts (double-buffered so
        # proj group k+1 matmuls overlap group k's bias-add drain), tag "o"
        # 2 bufs x 1 bank for the o accumulators = 8 banks total.
        ps_sc = ctx.enter_context(tc.tile_pool(name="ps_sc", bufs=2, space="PSUM"))
        ps_pp = ctx.enter_context(tc.tile_pool(name="ps_pp", bufs=2, space="PSUM"))
        ps_o = ctx.enter_context(tc.tile_pool(name="ps_o", bufs=2, space="PSUM"))

        # ---------------- load phase (consolidated DMAs) --------------------
        wq_sb = weights.tile([P, NDC, D], f16, tag="wq")
        wk_sb = weights.tile([P, NDC, D], f16, tag="wk")
        wv_sb = weights.tile([P, NDC, D], f16, tag="wv")
        qt_sb = raw.tile([P, NDC, ROWS], f16, tag="qt")
        bqk_sb = weights.tile([P, 2 * NPAIR], f32, tag="bqk")
        # DMA issue is ~1us of engine time per descriptor batch; spread the
        # load DMAs across four engines so issue itself doesn't serialize.
        nc.sync.dma_start(out=wk_sb, in_=wk_d)
        nc.gpsimd.dma_start(out=qt_sb, in_=qt_d)
        nc.scalar.dma_start(out=wq_sb, in_=wq_d)
        nc.scalar.dma_start(out=bqk_sb, in_=bqk_d)
        kt_slabs = []
        for g in range(S // 512):
            kt_slab = raw.tile([P, NDC, 512], f16, tag=f"kt{g}", name=f"kt_slab{g}")
            nc.sync.dma_start(out=kt_slab, in_=kt_d[g])
            kt_slabs.append(kt_slab)
        nc.gpsimd.dma_start(out=wv_sb, in_=wv_d)
        vt_q = []
        for q in range(NTT // 4):
            vq = raw.tile([P, 4, NDC, 128], f16, tag=f"vt{q}", name=f"vt_q{q}")
            nc.gpsimd.dma_start(out=vq, in_=vt_d[q])
            vt_q.append(vq)
        wo_sb = weights.tile([DV, H, D], f16, tag="wo")
        bob_sb = weights.tile([P, D], f32, tag="bob")
        nc.gpsimd.dma_start(out=wo_sb, in_=wo_d)
        nc.gpsimd.dma_start(out=bob_sb, in_=bob_d)
        ones64 = weights.tile([VW, DV], f16, tag="ones64")
        nc.vector.memset(ones64, 1.0)
        # dummy exp at t=0 pulls the ~2.7us ACT_TABLE_LOAD into the DMA wait
        warm = weights.tile([1, 16], f32, tag="warm")
        nc.vector.memset(warm, 0.0)
        warm2 = weights.tile([1, 16], f16, tag="warm2")
        nc.scalar.activation(warm2, warm, Exp)

        def vt_slab(t):
            return vt_q[t // 4][:, t % 4]

        # -------------- compute phase (optionally looped for bench) ---------
        loop_cm = (
            tc.For_i(
                0, hw_loop, 1, name="bench",
                hint_engines=(
                    mybir.EngineType.PE,
                    mybir.EngineType.Activation,
                    mybir.EngineType.DVE,
                    mybir.EngineType.SP,
                ),
            )
            if hw_loop
            else contextlib.nullcontext()
        )
        with loop_cm:
          for _rep in range(repeats):
            KT = [acts.tile([P, S], f16, tag=f"KT{p}", name=f"KT{p}") for p in range(NPAIR)]
            QT = [acts.tile([P, ROWS], f16, tag=f"QT{p}", name=f"QT{p}") for p in range(NPAIR)]
            Vp = [
                acts.tile([P, H, VW], f16, tag=f"Vp{t}", name=f"Vp{t}")
                for t in range(NTT)
            ]
            o2T = [acts.tile([DV, ROWS], f16, tag=f"o2T{i}", name=f"o2T{i}") for i in range(H)]
            den64 = acts.tile([VW, H, ROWS], f16, tag="den64", name="den64")

            def proj_qt(p):
                ps = ps_pp.tile([P, ROWS], f32, tag="pp", name="ps_q")
                for c in range(NDC):
                    nc.tensor.matmul(
                        ps, lhsT=wq_sb[:, c, p * 128 : (p + 1) * 128],
                        rhs=qt_sb[:, c, :],
                        start=(c == 0), stop=(c == NDC - 1),
                    )
                nc.vector.tensor_scalar_add(QT[p], ps, bqk_sb[:, p : p + 1])

            def proj_kt(p, g):
                ps = ps_pp.tile([P, 512], f32, tag="pp", name="ps_k")
                for c in range(NDC):
                    nc.tensor.matmul(
                        ps, lhsT=wk_sb[:, c, p * 128 : (p + 1) * 128],
                        rhs=kt_slabs[g][:, c, :],
                        start=(c == 0), stop=(c == NDC - 1),
                    )
                nc.vector.tensor_scalar_add(
                    KT[p][:, g * 512 : (g + 1) * 512], ps,
                    bqk_sb[:, NPAIR + p : NPAIR + p + 1],
                )

            def proj_v(t):
                ps = ps_pp.tile([P, D], f32, tag="pp", name="ps_v")
                for c in range(NDC):
                    nc.tensor.matmul(
                        ps, lhsT=vt_slab(t)[:, c, :], rhs=wv_sb[:, c, :],
                        start=(c == 0), stop=(c == NDC - 1),
                    )
                nc.vector.memset(Vp[t][:, :, DV : DV + 1], 1.0)
                nc.vector.tensor_copy(
                    Vp[t][:, :, 0:DV], ps.rearrange("p (i v) -> p i v", i=H)
                )

            def scores1(p, t):
                # scores for pair p, key tile t; one N=1024 exp (2 banks)
                ps = ps_sc.tile([P, 2, 512], f32, tag="sc", name="ps_sc_t")
                ts = slice(t * 128, (t + 1) * 128)
                for i in range(2):
                    nc.tensor.matmul(
                        ps[:, i, :],
                        lhsT=KT[p][64 * i : 64 * i + 64, ts],
                        rhs=QT[p][64 * i : 64 * i + 64, :],
                        start=True, stop=True,
                    )
                at = attn_pool.tile([P, 2, 512], f16, tag="at", name="at_t")
                nc.scalar.activation(at, ps, Exp, scale=1.0 / np.sqrt(DK))
                return at

            def ov_step1(p, o_ps, at, t):
                for i in range(2):
                    nc.tensor.matmul(
                        o_ps[i], lhsT=Vp[t][:, 2 * p + i, :], rhs=at[:, i, :],
                        start=(t == 0), stop=(t == NTT - 1),
                    )

            def den_copy(p, o_ps):
                for i in range(2):
                    nc.vector.tensor_copy(
                        den64[DV : DV + 1, 2 * p + i, :], o_ps[i][DV : DV + 1, :]
                    )

            def ov_finish(p, o_ps):
                # rows 0:64 = unnormalized head output, row 64 = softmax denom
                for i in range(2):
                    bc = ps_pp.tile([DV, 512], f32, tag="pp", name="bc_ps")
                    nc.tensor.matmul(
                        bc, lhsT=ones64[DV : DV + 1, :],
                        rhs=den64[DV : DV + 1, 2 * p + i, :], start=True, stop=True,
                    )
                    rbc = small.tile([DV, 512], f32, tag="rbc", name="rbc")
                    nc.vector.reciprocal_approx_fast(rbc, bc)
                    nc.vector.tensor_mul(o2T[2 * p + i], o_ps[i][0:DV, :], rbc)

            # Projection work interleaved into the attention rounds, keyed by
            # (pair, t). KT[p]/QT[p] must complete before pair p's scores.
            interleave = {
                (0, 0): [("v", 2)], (0, 1): [("v", 3), ("kt", 0, 2)],
                (0, 2): [("v", 4)], (0, 3): [("v", 5)],
                (0, 4): [("v", 6), ("kt", 0, 3)], (0, 5): [("v", 7)],
                (0, 6): [("v", 8)], (0, 7): [("v", 9)],
                (0, 8): [("v", 10)], (0, 9): [("v", 11)],
                (0, 10): [("v", 12), ("kt", 1, 0)], (0, 11): [("v", 13)],
                (0, 12): [("v", 14)], (0, 13): [("v", 15)],
                (0, 14): [("qt", 1)], (0, 15): [("kt", 2, 0)],
                (1, 0): [("kt", 1, 1)], (1, 1): [("kt", 2, 1)],
                (1, 2): [("kt", 1, 2)], (1, 3): [("kt", 2, 2)],
                (1, 4): [("kt", 1, 3)], (1, 5): [("kt", 2, 3)],
                (1, 7): [("kt", 3, 0)], (1, 9): [("kt", 3, 1)],
                (1, 11): [("qt", 2)], (1, 13): [("kt", 3, 2)],
                (1, 15): [("kt", 3, 3)],
                (2, 1): [("qt", 3)],
            }

            def do_interleave(p, t):
                for item in interleave.get((p, t), []):
                    if item[0] == "v":
                        proj_v(item[1])
                    elif item[0] == "kt":
                        proj_kt(item[1], item[2])
                    else:
                        proj_qt(item[1])

            # --- phase A: prologue for pair 0 + two V lead tiles (more V here
            # --- would stall the FIFO on the vt DMAs; the rest stream in
            # --- just-in-time via the interleave)
            proj_qt(0)
            proj_kt(0, 0)
            proj_kt(0, 1)
            proj_v(0)
            proj_v(1)

            # --- attention rounds, o-matmuls lag scores by one round so they
            # --- never stall the PE FIFO on the exp. Pair normalization is
            # --- issued one round after the pair's last o accumulation and
            # --- MUST precede the next pair's second o-step (its o-bank reuse
            # --- waits on norm work that would otherwise sit later in the PE
            # --- FIFO behind the stalled matmul).
            rounds = [(p, t) for p in range(NPAIR) for t in range(NTT)]
            o_ps_by_pair = {}
            pend = []
            for p, t in rounds:
                if t == 0:
                    o_ps_by_pair[p] = [
                        ps_o.tile([VW, ROWS], f32, tag="o", name=f"o_ps{i}")
                        for i in range(2)
                    ]
                at = scores1(p, t)
                pend.append((p, t, at))
                while len(pend) > 2:
                    # o-matmuls lag scores by two rounds so they never stall
                    # the in-order PE FIFO on an exp, even in rounds loaded
                    # with projection work
                    pp, pt, pat = pend.pop(0)
                    ov_step1(pp, o_ps_by_pair[pp], pat, pt)
                    if pt == NTT - 1:
                        den_copy(pp, o_ps_by_pair[pp])
                        ov_finish(pp, o_ps_by_pair[pp])
                do_interleave(p, t)
            while pend:
                pp, pt, pat = pend.pop(0)
                ov_step1(pp, o_ps_by_pair[pp], pat, pt)
                if pt == NTT - 1:
                    den_copy(pp, o_ps_by_pair[pp])
                    ov_finish(pp, o_ps_by_pair[pp])

            # --- output projection for this core's 512 rows
            for st in range(ROWS // P):
                ps = ps_pp.tile([P, D], f32, tag="pp", name="ps_out")
                for i in range(H):
                    nc.tensor.matmul(
                        ps, lhsT=o2T[i][:, st * 128 : (st + 1) * 128],
                        rhs=wo_sb[:, i, :],
                        start=(i == 0), stop=(i == H - 1),
                    )
                ot = small.tile([P, D], f32, tag="ot")
                nc.vector.tensor_add(ot, ps, bob_sb)
                nc.gpsimd.dma_start(out=out_d[st], in_=ot)

    nc.compile()
    return nc


def _get_program(repeats=1, hw_loop=0):
    key = (repeats, hw_loop)
    if key not in _prog:
        _prog[key] = _build_program(repeats=repeats, hw_loop=hw_loop)
    return _prog[key]


def _stage_inputs(queries, keys, values, wq, bq, wk, bk, wv, bv, wo, bo):
    """Host staging: transpose activations to [D, S], chunk weights, slice
    per-core query shards. Returns the 8 per-core input dicts."""
    h = np.float16
    qT = queries.transpose(0, 2, 1).astype(h)
    kT = keys.transpose(0, 2, 1).astype(h)
    vT = values.transpose(0, 2, 1).astype(h)

    def chunk(m):
        # [512, X] -> [128, NDC, X]: row c*128+p -> [p, c, :]
        return np.ascontiguousarray(m.reshape(NDC, P, m.shape[1]).transpose(1, 0, 2))

    wq_m = chunk(np.concatenate([wq[i] for i in range(H)], axis=1)).astype(h)
    wk_m = chunk(np.concatenate([wk[i] for i in range(H)], axis=1)).astype(h)
    wv_m = chunk(np.concatenate([wv[i] for i in range(H)], axis=1)).astype(h)
    wo_m = np.ascontiguousarray(wo.reshape(H, DV, D).transpose(1, 0, 2)).astype(h)
    bqk = np.concatenate(
        [bq.reshape(NPAIR, P).T, bk.reshape(NPAIR, P).T], axis=1
    ).astype(np.float32)
    bqk = np.ascontiguousarray(bqk)
    # fold bv through the output projection: out += concat(bv) @ wo
    bo_eff = (bo + bv.reshape(D) @ wo).astype(np.float32)
    bob = np.broadcast_to(bo_eff.reshape(1, D), (P, D)).astype(np.float32).copy()

    # kt slab layout [g, p, c, x]: kt[g,p,c,x] = kT[b][c*128+p, g*512+x]
    kt_b = [
        np.ascontiguousarray(kT[b].reshape(NDC, P, S // 512, 512).transpose(2, 1, 0, 3))
        for b in range(B)
    ]
    # vt layout [q, p, u, c, x]: tile t=4q+u; vt[...] = vT[b][c*128+p, t*128+x]
    vt_b = [
        np.ascontiguousarray(
            vT[b].reshape(NDC, P, NTT // 4, 4, 128).transpose(2, 1, 3, 0, 4)
        )
        for b in range(B)
    ]
    in_maps = []
    for c in range(NCORES):
        b, r = c // 4, c % 4
        qt_c = np.ascontiguousarray(
            qT[b][:, r * ROWS : (r + 1) * ROWS].reshape(NDC, P, ROWS).transpose(1, 0, 2)
        )
        in_maps.append(
            {
                "qt": qt_c,
                "kt": kt_b[b],
                "vt": vt_b[b],
                "wq": wq_m, "wk": wk_m, "wv": wv_m, "wo": wo_m,
                "bqk": bqk, "bob": bob,
            }
        )
    return in_maps


def run(trace=False, repeats=1, hw_loop=0, **inputs):
    """Run the kernel; returns (output, BassKernelResults)."""
    from concourse.bass_utils import run_bass_kernel_spmd

    nc = _get_program(repeats, hw_loop)
    in_maps = _stage_inputs(**inputs)
    res = run_bass_kernel_spmd(nc, in_maps, core_ids=list(range(NCORES)), trace=trace)
    out = np.empty((B, S, D), np.float32)
    for c in range(NCORES):
        b, r = c // 4, c % 4
        out[b, r * ROWS : (r + 1) * ROWS, :] = res.results[c]["out"].reshape(ROWS, D)
    return out, res


def kernel(**inputs):
    out, _ = run(trace=False, **inputs)
    return out


# revision 29
# speedup vs baseline: 1.0340x; 1.0077x over previous
"""Multi-head attention kernel for 8 Trainium2 NeuronCores.

Problem: B=2, S=2048, H=8, DK=DV=64, D=512 (nn_MultiHeadAttention).

Sharding: core c owns batch b=c//4 and query rows [512*r, 512*r+512) with
r = c%4. No collectives: each core recomputes the full K/V projections for
its batch locally (the ~25us of redundant PE work is far cheaper than the
barrier + AllGather latency it replaces).

Per-core device kernel, software-pipelined so the PE FIFO never waits on
the exp: round k issues scores(k) then the o-matmuls of round k-1, so each
o only executes after its attention tile finished exp ~1 round earlier.
  QT[p]   = wq2[p].T @ qT + bq              [128, 512]
  KT[p]   = wk2[p].T @ kT + bk              [128, 2048]
  V'[t]   = vT(t).T @ wv | ones col         [128, 8, 65]  (65th column of
            ones makes the o-matmul emit the softmax denominator in row 64)
  scoresT = KT[p] halves @ QT[p]            [128, 2, 512] per (pair, t),
                                            2 concurrent row-group matmuls
  attnT   = exp(scoresT / 8)                ScalarE, f16, no max-subtract
  o65[h] += V'[t,h] @ attnT[:, i]           accumulated over t; row 64 =
                                            softmax denominator
  bc      = ones(1x64).T @ denom row        K=1 matmul partition-broadcast
  rbc     = reciprocal_approx_fast(bc)      one DVE op per pair
  o2T[h]  = o65[h][0:64] * rbc              DVE, f16
  out     = sum_h o2T[h].T-slices @ wo[h] + bo'
bv is folded into the output bias on the host (bo' = bo + concat(bv) @ wo),
so the V projection needs no bias add on device.
"""

import numpy as np

B, S, H, DK, DV = 2, 2048, 8, 64, 64
D = H * DV  # 512
NCORES = 8
GROUP = 4  # cores per batch
ROWS = (B * S) // NCORES  # 512 query rows per core
NPAIR = H // 2  # 4 head pairs
NTT = S // 128  # 16 key/value tiles
NDC = D // 128  # 4 contraction chunks
P = 128
VW = DV + 1  # 65: V columns per head incl. the ones column

_prog = {}


def _build_program(repeats=1, hw_loop=0):
    from contextlib import ExitStack
    import contextlib

    import concourse.mybir as mybir
    import concourse.tile as tile
    from concourse import bacc

    f32 = mybir.dt.float32
    f16 = mybir.dt.float16  # fp16 PE datapath: separate+fast weight loads
    Exp = mybir.ActivationFunctionType.Exp

    nc = bacc.Bacc("TRN2", target_bir_lowering=False, debug=False, num_devices=NCORES)

    # DRAM I/O (per-core data; same program on all 8 cores)
    qt_d = nc.dram_tensor("qt", [P, NDC, ROWS], f16, kind="ExternalInput").ap()
    kt_d = nc.dram_tensor("kt", [S // 512, P, NDC, 512], f16, kind="ExternalInput").ap()
    vt_d = nc.dram_tensor("vt", [NTT // 4, P, 4, NDC, 128], f16, kind="ExternalInput").ap()
    wq_d = nc.dram_tensor("wq", [P, NDC, D], f16, kind="ExternalInput").ap()
    wk_d = nc.dram_tensor("wk", [P, NDC, D], f16, kind="ExternalInput").ap()
    wv_d = nc.dram_tensor("wv", [P, NDC, D], f16, kind="ExternalInput").ap()
    wo_d = nc.dram_tensor("wo", [DV, H, D], f16, kind="ExternalInput").ap()
    bqk_d = nc.dram_tensor("bqk", [P, 2 * NPAIR], f32, kind="ExternalInput").ap()
    bob_d = nc.dram_tensor("bob", [P, D], f32, kind="ExternalInput").ap()
    out_d = nc.dram_tensor("out", [ROWS // P, P, D], f32, kind="ExternalOutput").ap()

    with tile.TileContext(nc) as tc, ExitStack() as ctx:
        weights = ctx.enter_context(tc.tile_pool(name="weights", bufs=1))
        raw = ctx.enter_context(tc.tile_pool(name="raw", bufs=1))
        acts = ctx.enter_context(tc.tile_pool(name="acts", bufs=1))
        attn_pool = ctx.enter_context(tc.tile_pool(name="attn", bufs=4))
        small = ctx.enter_context(tc.tile_pool(name="small", bufs=2))
        # PSUM: tag "sc" 2 bufs x [128,2,512]f32 (2 banks each) dedicated to
        # scores (strict double-buffer against the exp), tag "pp" 2 bufs x
        # 1 bank for projection groups + denom broadcasts (double-buffered so
        # proj group k+1 matmuls overlap group k's bias-add drain), tag "o"
        # 2 bufs x 1 bank for the o accumulators = 8 banks total.
        ps_sc = ctx.enter_context(tc.tile_pool(name="ps_sc", bufs=2, space="PSUM"))
        ps_pp = ctx.enter_context(tc.tile_pool(name="ps_pp", bufs=2, space="PSUM"))
        ps_o = ctx.enter_context(tc.tile_pool(name="ps_o", bufs=2, space="PSUM"))

        # ---------------- load phase (consolidated DMAs) --------------------
        wq_sb = weights.tile([P, NDC, D], f16, tag="wq")
        wk_sb = weights.tile([P, NDC, D], f16, tag="wk")
        wv_sb = weights.tile([P, NDC, D], f16, tag="wv")
        qt_sb = raw.tile([P, NDC, ROWS], f16, tag="qt")
        bqk_sb = weights.tile([P, 2 * NPAIR], f32, tag="bqk")
        # DMA issue is ~1us of engine time per descriptor batch; spread the
        # load DMAs across four engines so issue itself doesn't serialize.
        nc.sync.dma_start(out=wk_sb, in_=wk_d)
        nc.gpsimd.dma_start(out=qt_sb, in_=qt_d)
        nc.scalar.dma_start(out=wq_sb, in_=wq_d)
        nc.scalar.dma_start(out=bqk_sb, in_=bqk_d)
        kt_slabs = []
        for g in range(S // 512):
            kt_slab = raw.tile([P, NDC, 512], f16, tag=f"kt{g}", name=f"kt_slab{g}")
            nc.sync.dma_start(out=kt_slab, in_=kt_d[g])
            kt_slabs.append(kt_slab)
        nc.gpsimd.dma_start(out=wv_sb, in_=wv_d)
        vt_q = []
        for q in range(NTT // 4):
            vq = raw.tile([P, 4, NDC, 128], f16, tag=f"vt{q}", name=f"vt_q{q}")
            nc.gpsimd.dma_start(out=vq, in_=vt_d[q])
            vt_q.append(vq)
        wo_sb = weights.tile([DV, H, D], f16, tag="wo")
        bob_sb = weights.tile([P, D], f32, tag="bob")
        nc.gpsimd.dma_start(out=wo_sb, in_=wo_d)
        nc.gpsimd.dma_start(out=bob_sb, in_=bob_d)
        ones64 = weights.tile([VW, DV], f16, tag="ones64")
        nc.vector.memset(ones64, 1.0)
        # dummy exp at t=0 pulls the ~2.7us ACT_TABLE_LOAD into the DMA wait
        warm = weights.tile([1, 16], f32, tag="warm")
        nc.vector.memset(warm, 0.0)
        warm2 = weights.tile([1, 16], f16, tag="warm2")
        nc.scalar.activation(warm2, warm, Exp)

        def vt_slab(t):
            return vt_q[t // 4][:, t % 4]

        # -------------- compute phase (optionally looped for bench) ---------
        loop_cm = (
            tc.For_i(
                0, hw_loop, 1, name="bench",
                hint_engines=(
                    mybir.EngineType.PE,
                    mybir.EngineType.Activation,
                    mybir.EngineType.DVE,
                    mybir.EngineType.SP,
                ),
            )
            if hw_loop
            else contextlib.nullcontext()
        )
        with loop_cm:
          for _rep in range(repeats):
            KT = [acts.tile([P, S], f16, tag=f"KT{p}", name=f"KT{p}") for p in range(NPAIR)]
            QT = [acts.tile([P, ROWS], f16, tag=f"QT{p}", name=f"QT{p}") for p in range(NPAIR)]
            Vp = [
                acts.tile([P, H, VW], f16, tag=f"Vp{t}", name=f"Vp{t}")
                for t in range(NTT)
            ]
            o2T = [acts.tile([DV, ROWS], f16, tag=f"o2T{i}", name=f"o2T{i}") for i in range(H)]
            den64 = acts.tile([VW, H, ROWS], f16, tag="den64", name="den64")

            def proj_qt(p):
                ps = ps_pp.tile([P, ROWS], f32, tag="pp", name="ps_q")
                for c in range(NDC):
                    nc.tensor.matmul(
                        ps, lhsT=wq_sb[:, c, p * 128 : (p + 1) * 128],
                        rhs=qt_sb[:, c, :],
                        start=(c == 0), stop=(c == NDC - 1),
                    )
                nc.vector.tensor_scalar_add(QT[p], ps, bqk_sb[:, p : p + 1])

            def proj_kt(p, g):
                ps = ps_pp.tile([P, 512], f32, tag="pp", name="ps_k")
                for c in range(NDC):
                    nc.tensor.matmul(
                        ps, lhsT=wk_sb[:, c, p * 128 : (p + 1) * 128],
                        rhs=kt_slabs[g][:, c, :],
                        start=(c == 0), stop=(c == NDC - 1),
                    )
                nc.vector.tensor_scalar_add(
                    KT[p][:, g * 512 : (g + 1) * 512], ps,
                    bqk_sb[:, NPAIR + p : NPAIR + p + 1],
                )

            def proj_v(t):
                ps = ps_pp.tile([P, D], f32, tag="pp", name="ps_v")
                for c in range(NDC):
                    nc.tensor.matmul(
                        ps, lhsT=vt_slab(t)[:, c, :], rhs=wv_sb[:, c, :],
                        start=(c == 0), stop=(c == NDC - 1),
                    )
                nc.vector.memset(Vp[t][:, :, DV : DV + 1], 1.0)
                nc.vector.tensor_copy(
                    Vp[t][:, :, 0:DV], ps.rearrange("p (i v) -> p i v", i=H)
                )

            def scores1(p, t):
                # scores for pair p, key tile t; one N=1024 exp (2 banks)
                ps = ps_sc.tile([P, 2, 512], f32, tag="sc", name="ps_sc_t")
                ts = slice(t * 128, (t + 1) * 128)
                for i in range(2):
                    nc.tensor.matmul(
                        ps[:, i, :],
                        lhsT=KT[p][64 * i : 64 * i + 64, ts],
                        rhs=QT[p][64 * i : 64 * i + 64, :],
                        start=True, stop=True,
                    )
                at = attn_pool.tile([P, 2, 512], f16, tag="at", name="at_t")
                nc.scalar.activation(at, ps, Exp, scale=1.0 / np.sqrt(DK))
                return at

            def ov_step1(p, o_ps, at, t):
                for i in range(2):
                    nc.tensor.matmul(
                        o_ps[i], lhsT=Vp[t][:, 2 * p + i, :], rhs=at[:, i, :],
                        start=(t == 0), stop=(t == NTT - 1),
                    )

            def den_copy(p, o_ps):
                for i in range(2):
                    nc.vector.tensor_copy(
                        den64[DV : DV + 1, 2 * p + i, :], o_ps[i][DV : DV + 1, :]
                    )

            def ov_finish(p, o_ps):
                # rows 0:64 = unnormalized head output, row 64 = softmax denom
                for i in range(2):
                    bc = ps_pp.tile([DV, 512], f32, tag="pp", name="bc_ps")
                    nc.tensor.matmul(
                        bc, lhsT=ones64[DV : DV + 1, :],
                        rhs=den64[DV : DV + 1, 2 * p + i, :], start=True, stop=True,
                    )
                    rbc = small.tile([DV, 512], f32, tag="rbc", name="rbc")
                    nc.vector.reciprocal_approx_fast(rbc, bc)
                    nc.vector.tensor_mul(o2T[2 * p + i], o_ps[i][0:DV, :], rbc)

            # Projection work interleaved into the attention rounds, keyed by
            # (pair, t). KT[p]/QT[p] must complete before pair p's scores.
            interleave = {
                (0, 0): [("v", 2)], (0, 1): [("v", 3), ("kt", 0, 2)],
                (0, 2): [("v", 4)], (0, 3): [("v", 5)],
                (0, 4): [("v", 6), ("kt", 0, 3)], (0, 5): [("v", 7)],
                (0, 6): [("v", 8), ("kt", 1, 0)], (0, 7): [("v", 9)],
                (0, 8): [("v", 10), ("kt", 1, 1)], (0, 9): [("v", 11)],
                (0, 10): [("v", 12), ("kt", 1, 2)], (0, 11): [("v", 13)],
                (0, 12): [("v", 14), ("kt", 1, 3)], (0, 13): [("v", 15)],
                (0, 14): [("qt", 1)], (0, 15): [("kt", 2, 0)],
                (1, 1): [("kt", 2, 1)], (1, 3): [("kt", 2, 2)],
                (1, 5): [("kt", 2, 3)], (1, 7): [("kt", 3, 0)],
                (1, 9): [("kt", 3, 1)], (1, 11): [("qt", 2)],
                (1, 13): [("kt", 3, 2)], (1, 15): [("kt", 3, 3)],
                (2, 1): [("qt", 3)],
            }

            def do_interleave(p, t):
                for item in interleave.get((p, t), []):
                    if item[0] == "v":
                        proj_v(item[1])
                    elif item[0] == "kt":
                        proj_kt(item[1], item[2])
                    else:
                        proj_qt(item[1])

            # --- phase A: prologue for pair 0 + two V lead tiles (more V here
            # --- would stall the FIFO on the vt DMAs; the rest stream in
            # --- just-in-time via the interleave)
            proj_qt(0)
            proj_kt(0, 0)
            proj_kt(0, 1)
            proj_v(0)
            proj_v(1)

            # --- attention rounds, o-matmuls lag scores by one round so they
            # --- never stall the PE FIFO on the exp. Pair normalization is
            # --- issued one round after the pair's last o accumulation and
            # --- MUST precede the next pair's second o-step (its o-bank reuse
            # --- waits on norm work that would otherwise sit later in the PE
            # --- FIFO behind the stalled matmul).
            rounds = [(p, t) for p in range(NPAIR) for t in range(NTT)]
            o_ps_by_pair = {}
            pend = []
            for p, t in rounds:
                if t == 0:
                    o_ps_by_pair[p] = [
                        ps_o.tile([VW, ROWS], f32, tag="o", name=f"o_ps{i}")
                        for i in range(2)
                    ]
                at = scores1(p, t)
                pend.append((p, t, at))
                while len(pend) > 2:
                    # o-matmuls lag scores by two rounds so they never stall
                    # the in-order PE FIFO on an exp, even in rounds loaded
                    # with projection work
                    pp, pt, pat = pend.pop(0)
                    ov_step1(pp, o_ps_by_pair[pp], pat, pt)
                    if pt == NTT - 1:
                        den_copy(pp, o_ps_by_pair[pp])
                        ov_finish(pp, o_ps_by_pair[pp])
                do_interleave(p, t)
            while pend:
                pp, pt, pat = pend.pop(0)
                ov_step1(pp, o_ps_by_pair[pp], pat, pt)
                if pt == NTT - 1:
                    den_copy(pp, o_ps_by_pair[pp])
                    ov_finish(pp, o_ps_by_pair[pp])

            # --- output projection for this core's 512 rows
            for st in range(ROWS // P):
                ps = ps_pp.tile([P, D], f32, tag="pp", name="ps_out")
                for i in range(H):
                    nc.tensor.matmul(
                        ps, lhsT=o2T[i][:, st * 128 : (st + 1) * 128],
                        rhs=wo_sb[:, i, :],
                        start=(i == 0), stop=(i == H - 1),
                    )
                ot = small.tile([P, D], f32, tag="ot")
                nc.vector.tensor_add(ot, ps, bob_sb)
                nc.gpsimd.dma_start(out=out_d[st], in_=ot)

    nc.compile()
    return nc


def _get_program(repeats=1, hw_loop=0):
    key = (repeats, hw_loop)
    if key not in _prog:
        _prog[key] = _build_program(repeats=repeats, hw_loop=hw_loop)
    return _prog[key]


def _stage_inputs(queries, keys, values, wq, bq, wk, bk, wv, bv, wo, bo):
    """Host staging: transpose activations to [D, S], chunk weights, slice
    per-core query shards. Returns the 8 per-core input dicts."""
    h = np.float16
    qT = queries.transpose(0, 2, 1).astype(h)
    kT = keys.transpose(0, 2, 1).astype(h)
    vT = values.transpose(0, 2, 1).astype(h)

    def chunk(m):
        # [512, X] -> [128, NDC, X]: row c*128+p -> [p, c, :]
        return np.ascontiguousarray(m.reshape(NDC, P, m.shape[1]).transpose(1, 0, 2))

    wq_m = chunk(np.concatenate([wq[i] for i in range(H)], axis=1)).astype(h)
    wk_m = chunk(np.concatenate([wk[i] for i in range(H)], axis=1)).astype(h)
    wv_m = chunk(np.concatenate([wv[i] for i in range(H)], axis=1)).astype(h)
    wo_m = np.ascontiguousarray(wo.reshape(H, DV, D).transpose(1, 0, 2)).astype(h)
    bqk = np.concatenate(
        [bq.reshape(NPAIR, P).T, bk.reshape(NPAIR, P).T], axis=1
    ).astype(np.float32)
    bqk = np.ascontiguousarray(bqk)
    # fold bv through the output projection: out += concat(bv) @ wo
    bo_eff = (bo + bv.reshape(D) @ wo).astype(np.float32)
    bob = np.broadcast_to(bo_eff.reshape(1, D), (P, D)).astype(np.float32).copy()

    # kt slab layout [g, p, c, x]: kt[g,p,c,x] = kT[b][c*128+p, g*512+x]
    kt_b = [
        np.ascontiguousarray(kT[b].reshape(NDC, P, S // 512, 512).transpose(2, 1, 0, 3))
        for b in range(B)
    ]
    # vt layout [q, p, u, c, x]: tile t=4q+u; vt[...] = vT[b][c*128+p, t*128+x]
    vt_b = [
        np.ascontiguousarray(
            vT[b].reshape(NDC, P, NTT // 4, 4, 128).transpose(2, 1, 3, 0, 4)
        )
        for b in range(B)
    ]
    in_maps = []
    for c in range(NCORES):
        b, r = c // 4, c % 4
        qt_c = np.ascontiguousarray(
            qT[b][:, r * ROWS : (r + 1) * ROWS].reshape(NDC, P, ROWS).transpose(1, 0, 2)
        )
        in_maps.append(
            {
                "qt": qt_c,
                "kt": kt_b[b],
                "vt": vt_b[b],
                "wq": wq_m, "wk": wk_m, "wv": wv_m, "wo": wo_m,
                "bqk": bqk, "bob": bob,
            }
        )
    return in_maps


def run(trace=False, repeats=1, hw_loop=0, **inputs):
    """Run the kernel; returns (output, BassKernelResults)."""
    from concourse.bass_utils import run_bass_kernel_spmd

    nc = _get_program(repeats, hw_loop)
    in_maps = _stage_inputs(**inputs)
    res = run_bass_kernel_spmd(nc, in_maps, core_ids=list(range(NCORES)), trace=trace)
    out = np.empty((B, S, D), np.float32)
    for c in range(NCORES):
        b, r = c // 4, c % 4
        out[b, r * ROWS : (r + 1) * ROWS, :] = res.results[c]["out"].reshape(ROWS, D)
    return out, res


def kernel(**inputs):
    out, _ = run(trace=False, **inputs)
    return out
